# revision 11
# baseline (speedup 1.0000x reference)
"""CorrelationAwareFocalLoss on 8 trn2 NeuronCores.

Data-parallel over B (131072 -> 8 x 16384 rows). Each core computes,
over its shard (layout [128 partitions, 128 chunks x 64 cols]):
  z  = x*(1-2t);  sg = sigmoid(z);  spn = ln(1-sg) = -softplus(z)
  E' = sg^2 * spn          (= -focal term sans pos_weight)
  tp = (x>=0)*t
and accumulates via one matmul per 128-row chunk over the packed
[t | tp | E'] tile:
  out = [t|tp].T @ [t|tp|E']  ->  G, M1, M3, t.T@E'
plus per-partition row-sums of E'. Host sums per-core partials, builds
the thresholded correlation matrix A, and assembles the scalar loss.
"""

import numpy as np
import ml_dtypes

import concourse.bacc as bacc
import concourse.mybir as mybir
import concourse.tile as tile
from concourse.alu_op_type import AluOpType
from concourse.bass_utils import run_bass_kernel_spmd
import concourse.bass_utils as _bu
import bass_rust as _bass_rust

B, C = 131072, 64
N_CORES = 8
BS = B // N_CORES          # 16384 rows per core
P = 128                    # partitions
NCHUNK = BS // P           # 128 chunks of 128 rows
F = NCHUNK * C             # 8192 free columns per partition
NG = 4                     # pipeline groups
GS = F // NG               # 2048 free cols per group
CPG = GS // C              # 32 chunks per group
S = 3 * C                  # 192-col packed stride: [t | tp | E']

CORR_WEIGHT = 0.5
CORR_THRESH = 0.3

BF16 = mybir.dt.bfloat16
F32 = mybir.dt.float32


def build_nc():
    nc = bacc.Bacc(None, target_bir_lowering=False, debug=False)
    xb_d = nc.declare_dram_parameter("xb", [P, F], BF16, isOutput=False)
    tb_d = nc.declare_dram_parameter("tb", [P, F], BF16, isOutput=False)
    out_d = nc.declare_dram_parameter("out", [P, S + NG], F32, isOutput=True)

    with tile.TileContext(nc) as tc:
        with (
            tc.tile_pool(name="io", bufs=3) as io_pool,
            tc.tile_pool(name="pk", bufs=NG) as pk_pool,
            tc.tile_pool(name="sg", bufs=NG) as sg_pool,
            tc.tile_pool(name="mid", bufs=3) as mid_pool,
            tc.tile_pool(name="res", bufs=1) as res_pool,
            tc.tile_pool(name="psum", bufs=1, space="PSUM") as psum_pool,
        ):
            outt = res_pool.tile([P, S + NG], F32)
            psum = psum_pool.tile([P, S], F32)

            xs, tst, pks, zs, sgs, sps, sqs = [], [], [], [], [], [], []
            # phase 1: DMA in; z = x*(1-2t); tp = (x>=0)*t; pack t
            for g in range(NG):
                xg = io_pool.tile([P, GS], BF16)
                nc.gpsimd.dma_start(xg[:], xb_d[:, g * GS:(g + 1) * GS])
                tg = io_pool.tile([P, GS], BF16)
                nc.gpsimd.dma_start(tg[:], tb_d[:, g * GS:(g + 1) * GS])
                xs.append(xg)
                tst.append(tg)

                pkg = pk_pool.tile([P, CPG * S], BF16)  # [t | tp | E'] per chunk
                pk3 = pkg[:].rearrange("p (j f) -> p j f", f=S)
                t3 = tg[:].rearrange("p (j f) -> p j f", f=C)
                x3 = xg[:].rearrange("p (j f) -> p j f", f=C)
                pks.append(pkg)

                s1 = mid_pool.tile([P, GS], BF16)
                nc.vector.tensor_scalar(s1[:], tg[:], -2.0, 1.0,
                                        op0=AluOpType.mult, op1=AluOpType.add)
                zg = mid_pool.tile([P, GS], BF16)
                nc.vector.tensor_tensor(zg[:], xg[:], s1[:], op=AluOpType.mult)
                zs.append(zg)

                nc.vector.tensor_copy(pk3[:, :, 0:C], t3)
                pr = mid_pool.tile([P, GS], BF16)
                nc.vector.tensor_scalar(pr[:], xg[:], 0.0, None,
                                        op0=AluOpType.is_ge)
                p3 = pr[:].rearrange("p (j f) -> p j f", f=C)
                nc.vector.tensor_tensor(pk3[:, :, C:2 * C], p3, t3,
                                        op=AluOpType.mult)

            # phase 2: ACT sweeps batched per table set (2 loads total)
            sg_insts = []
            for g in range(NG):
                sgg = sg_pool.tile([P, GS], BF16)
                sg_insts.append(nc.scalar.activation(
                    sgg[:], zs[g][:], mybir.ActivationFunctionType.Sigmoid))
                sgs.append(sgg)
            # ln(1-sg) = ln(sigmoid(-z)) = -softplus(z); sign fixed on host
            for g in range(NG):
                spg = mid_pool.tile([P, GS], BF16)
                ln_inst = nc.scalar.activation(
                    spg[:], sgs[g][:], mybir.ActivationFunctionType.Ln,
                    scale=-1.0, bias=1.0)
                # keep all Sigmoids before any Ln: 2 table loads, not 4+
                _bass_rust.add_dep_helper(ln_inst.ins, sg_insts[-1].ins,
                                          reason="act table-set batching")
                sps.append(spg)

            # phase 3: E' = sg^2 * spn (+ row-sum accum); one matmul per chunk
            for g in range(NG):
                sq = mid_pool.tile([P, GS], BF16)
                nc.vector.tensor_tensor(sq[:], sgs[g][:], sgs[g][:],
                                        op=AluOpType.mult)
                pk3 = pks[g][:].rearrange("p (j f) -> p j f", f=S)
                s3 = sq[:].rearrange("p (j f) -> p j f", f=C)
                l3 = sps[g][:].rearrange("p (j f) -> p j f", f=C)
                nc.vector.scalar_tensor_tensor(
                    pk3[:, :, 2 * C:S], s3, 0.0, l3,
                    op0=AluOpType.add, op1=AluOpType.mult,
                    accum_out=outt[:, S + g:S + g + 1])

            for g in range(NG):
                for j in range(CPG):
                    first = g == 0 and j == 0
                    last = g == NG - 1 and j == CPG - 1
                    nc.tensor.matmul(psum[:],
                                     pks[g][:, j * S:j * S + 128],
                                     pks[g][:, j * S:(j + 1) * S],
                                     start=first, stop=last,
                                     skip_group_check=True)

            nc.vector.tensor_copy(outt[:, 0:S], psum[:])
            nc.gpsimd.dma_start(out_d[:], outt[:])
    nc.compile()
    return nc


_NC_CACHE = None


def _get_nc():
    global _NC_CACHE
    if _NC_CACHE is None:
        _NC_CACHE = build_nc()
    return _NC_CACHE


def _relayout(a: np.ndarray) -> np.ndarray:
    # [BS, C] -> [P, NCHUNK*C] with partition p, free = chunk*C + c
    a = a.reshape(NCHUNK, P, C).transpose(1, 0, 2)
    return np.ascontiguousarray(a).reshape(P, F)


def kernel(inputs: np.ndarray, targets: np.ndarray,
           pos_weights: np.ndarray) -> np.ndarray:
    nc = _get_nc()
    bf16 = ml_dtypes.bfloat16
    in_maps = []
    for k in range(N_CORES):
        sl = slice(k * BS, (k + 1) * BS)
        in_maps.append({
            "xb": _relayout(np.asarray(inputs[sl], np.float32)).astype(bf16),
            "tb": _relayout(np.asarray(targets[sl], np.float32)).astype(bf16),
        })
    res = run_bass_kernel_spmd(nc, in_maps, list(range(N_CORES)))

    o = np.zeros((P, S + NG), np.float64)
    for k in range(N_CORES):
        o += res.results[k]["out"].astype(np.float64)
    G = o[0:C, 0:C]
    M1 = o[C:128, 0:C]
    M3 = o[C:128, C:2 * C]
    # E' = -E: flip signs of the focal pieces
    D1 = -np.diag(o[0:C, 2 * C:S])
    S0 = -o[:, S:].sum()

    corr = G / B
    off = ~np.eye(C, dtype=bool)
    A = np.where((corr > CORR_THRESH) & off, corr, 0.0) * CORR_WEIGHT
    penalty_sum = (A * (M1 + M1.T - 2.0 * M3)).sum()
    w = np.asarray(pos_weights, np.float64)
    focal_sum = S0 + ((w - 1.0) * D1).sum()
    loss = (focal_sum + penalty_sum) / (B * C)
    return np.float32(loss)
